# revision 12
# baseline (speedup 1.0000x reference)
"""Trainium2 Bass kernel for nn_ExpertsChooseMaskedExpand (MoE routing).

Reference computes (per batch b):
    xd[e,c,j] = sum_t mask[t,e,c] * x[t,e,j]          (dispatch)
    y[e,c,o]  = sum_j xd[e,c,j] * w[e,o,j] + bias[o]  (expert GEMM)
    out[t,o]  = sum_{e,c} comb[t,e,c] * y[e,c,o]      (combine)

We use associativity to contract comb with xd first:
    z[t,e,j] = sum_c comb[t,e,c] * xd[e,c,j]
    out[t,o] = sum_{e,j} z[t,e,j] * w[e,o,j] + bias[o] * S[t],
    S[t] = sum_{e,c} comb[t,e,c]
which cuts FLOPs ~3.4x and never materializes y (B,E,C,O).

Sharding: 8 cores; core k handles batch b=k//2, token half h=k%2 (2048
tokens). Dispatch needs the whole batch's tokens; a pure token-split
would need collectives for every expert and a measured pair-wise
256KB AllReduce costs ~30us (latency-bound, starved while input DMA
streams). So we hybridize:
  - experts 0-3: dispatch duplicated across the pair (each core loads
    the full batch's mask slice) -> xd ready early with no collective.
  - experts 4-7: token-split partial dispatch + ONE pair-wise
    AllReduce(add) of the partial xd (512KB), whose ~30us latency is
    hidden under z-stage + combine pass-1 work for experts 0-3.
The combine for the first token chunk-group runs as two passes
(experts 0-3 into a bf16 partial while waiting, experts 4-7 merged
with a vector add) so the PE never idles at the collective gate.

All matmuls run in bf16 with fp32 PSUM accumulation; inputs are
cast/re-laid-out on host. Graph is SPMD-uniform (the AllReduce always
carries global experts 4-7; both cores' halves sum identically).
"""

import numpy as np
import ml_dtypes

BF16 = ml_dtypes.bfloat16

B, T, E, C = 4, 4096, 8, 512
I = 128            # per-expert input features
O = 4096           # out_features
NCORES = 8
TLOC = B * T // NCORES      # 2048 tokens per core
NTT = T // 128              # 32 token tiles, full batch (dup dispatch)
NTTL = TLOC // 128          # 16 local token tiles (split dispatch)
NCT = C // 128              # 4 c-blocks
NTC = TLOC // 512           # 4 t-chunks (z stage)
NOT = O // 512              # 8 o-tiles

_CACHE = {}


def _build():
    import concourse.bass as bass
    import concourse.tile as tile
    import concourse.bacc as bacc
    import concourse.mybir as mybir

    f32 = mybir.dt.float32
    bf16 = mybir.dt.bfloat16
    ts = bass.ts

    nc = bacc.Bacc(None, target_bir_lowering=False, debug=False)

    # experts 0-3: full batch (dup); experts 4-7: local token half
    xhd = nc.dram_tensor("xhd", [4, 128, NTT, I], bf16, kind="ExternalInput")
    mhd = nc.dram_tensor("mhd", [4, 128, NTT, C], bf16, kind="ExternalInput")
    xhs = nc.dram_tensor("xhs", [4, 128, NTTL, I], bf16,
                         kind="ExternalInput")
    mhs = nc.dram_tensor("mhs", [4, 128, NTTL, C], bf16,
                         kind="ExternalInput")
    cbt = nc.dram_tensor("cbt", [E, NCT, 128, TLOC], bf16,
                         kind="ExternalInput")
    wf = nc.dram_tensor("wf", [128, E, O], bf16, kind="ExternalInput")
    ident = nc.dram_tensor("ident", [128, 128], bf16, kind="ExternalInput")
    out_d = nc.dram_tensor("out", [TLOC, O], f32, kind="ExternalOutput")

    groups = [[0, 1], [2, 3], [4, 5], [6, 7]]

    with tile.TileContext(nc) as tc:
        with (
            tc.tile_pool(name="persist", bufs=1) as persist,
            tc.tile_pool(name="stream", bufs=1) as stream,
            tc.tile_pool(name="psum", bufs=1, space="PSUM") as psum,
            tc.tile_pool(name="dram", bufs=1, space="DRAM") as dram,
        ):
            wf_sb = persist.tile([128, E, O], bf16, tag="wf")
            id_sb = persist.tile([128, 128], bf16, tag="ident")
            nc.scalar.dma_start(id_sb[:], ident[:])

            cc_in = dram.tile([4, 128, NCT, 128], bf16, name="ccin")
            cc_out = dram.tile([4, 128, NCT, 128], bf16, name="ccout")

            xd = {}   # e -> xd tile [128c, NCT, 128j] bf16
            zt = {}   # (e, tch) -> z^T tile [128j, 512t] bf16
            po = {}   # (tt, ot) -> bf16 pass-1 partial (experts 0-3)

            def dispatch(e, xsrc, msrc, ei, ntt, to_cc):
                """xd^T psum accumulation for expert e over ntt token
                tiles, then transpose to [c, j]. to_cc: DMA partial to
                the collective input instead of keeping locally."""
                ps_a = psum.tile([128, C], f32, tag="psA", bufs=2,
                                 name=f"psA{e}")
                for q0 in range(0, ntt, 8):
                    mh_t = stream.tile([128, 8, C], bf16, tag="mh", bufs=3,
                                       name=f"mh{e}_{q0}")
                    nc.sync.dma_start(mh_t[:], msrc[ei, :, q0:q0 + 8, :])
                    xh_t = stream.tile([128, 8, I], bf16, tag="xh", bufs=3,
                                       name=f"xh{e}_{q0}")
                    nc.scalar.dma_start(xh_t[:], xsrc[ei, :, q0:q0 + 8, :])
                    for i in range(8):
                        tt = q0 + i
                        nc.tensor.matmul(
                            ps_a[:],
                            xh_t[:, i, :],
                            mh_t[:, i, :],
                            start=(tt == 0),
                            stop=(tt == ntt - 1),
                        )
                xdt = stream.tile([128, C], bf16, tag="xdt", bufs=2,
                                  name=f"xdt{e}")
                nc.vector.tensor_copy(xdt[:], ps_a[:])
                xdp = stream.tile([128, NCT, 128], bf16, tag="xdp", bufs=2,
                                  name=f"xdp{e}") if to_cc else \
                    persist.tile([128, NCT, 128], bf16, tag=f"xd{e}",
                                 name=f"xd{e}")
                for cb in range(NCT):
                    ps_t = psum.tile([128, 128], bf16, tag="psT", bufs=2,
                                     name=f"psT{e}_{cb}")
                    nc.tensor.transpose(ps_t[:],
                                        xdt[:, ts(cb, 128)], id_sb[:])
                    nc.vector.tensor_copy(xdp[:, cb, :], ps_t[:])
                if to_cc:
                    nc.scalar.dma_start(cc_in[e - 4], xdp[:])
                else:
                    xd[e] = xdp

            def zstage(e, tch):
                cb_t = stream.tile([128, NCT, 512], bf16, tag="cb", bufs=4,
                                   name=f"cb{e}_{tch}")
                for cb in range(NCT):
                    nc.sync.dma_start(cb_t[:, cb, :],
                                      cbt[e, cb, :, ts(tch, 512)])
                ps_z = psum.tile([128, 512], f32, tag="psZ", bufs=2,
                                 name=f"psZ{e}_{tch}")
                for cb in range(NCT):
                    nc.tensor.matmul(
                        ps_z[:],
                        xd[e][:, cb, :],
                        cb_t[:, cb, :],
                        start=(cb == 0),
                        stop=(cb == NCT - 1),
                    )
                z_sb = persist.tile([128, 512], bf16, tag=f"zt{e}_{tch}",
                                    name=f"zt{e}_{tch}")
                nc.vector.tensor_copy(z_sb[:], ps_z[:])
                zt[(e, tch)] = z_sb

            def pass1(tt, ot):
                """Combine partial over experts 0-3 -> bf16 tile."""
                tch, m = tt // 4, tt % 4
                ps_p = psum.tile([128, 512], f32, tag="psC", bufs=2,
                                 name=f"psP{tt}_{ot}")
                for e in range(4):
                    nc.tensor.matmul(
                        ps_p[:],
                        zt[(e, tch)][:, ts(m, 128)],
                        wf_sb[:, e, ts(ot, 512)],
                        start=(e == 0),
                        stop=(e == 3),
                    )
                pb = persist.tile([128, 512], bf16, tag=f"po{tt}_{ot}",
                                  name=f"po{tt}_{ot}")
                nc.vector.tensor_copy(pb[:], ps_p[:])
                po[(tt, ot)] = pb

            # ---- Own phase ----
            # experts 0-3 over the full batch (xd complete, no cc);
            # experts 4-7 partial over the local half, one AllReduce.
            for e in range(4):
                dispatch(e, xhd, mhd, e, NTT, False)
            for e in range(4, 8):
                dispatch(e, xhs, mhs, e - 4, NTTL, True)
            nc.gpsimd.collective_compute(
                "AllReduce",
                mybir.AluOpType.add,
                replica_groups=groups,
                ins=[cc_in[:].opt()],
                outs=[cc_out[:].opt()],
            )
            for e in range(4, 8):
                xr = persist.tile([128, NCT, 128], bf16, tag=f"xd{e}",
                                  name=f"xd{e}")
                nc.gpsimd.dma_start(xr[:], cc_out[e - 4])
                xd[e] = xr

            # weights stream behind the dispatch loads
            for e in range(E):
                nc.scalar.dma_start(wf_sb[:, e, :], wf[:, e, :])

            # fill while the collective is in flight: z for experts 0-3,
            # then combine pass-1 (experts 0-3) for chunk-groups 0-1
            for tch in range(NTC):
                for e in range(4):
                    zstage(e, tch)
            for tt in range(4):
                for ot in range(NOT):
                    pass1(tt, ot)
            # gate: z for experts 4-7 (needs the AllReduce result)
            for tch in range(NTC):
                for e in range(4, 8):
                    zstage(e, tch)

            # ---- Combine phase ----
            for tt in range(NTTL):
                tch, m = tt // 4, tt % 4
                two_pass = tt < 4
                e0 = 4 if two_pass else 0
                out_sb = stream.tile([128, O // 2], f32, tag="out",
                                     bufs=2, name=f"out{tt}")
                for ot in range(NOT):
                    if ot == NOT // 2:
                        nc.scalar.dma_start(
                            out_d[ts(tt, 128), 0:O // 2], out_sb[:])
                        out_sb = stream.tile([128, O // 2], f32,
                                             tag="out", bufs=2,
                                             name=f"out{tt}b")
                    ps_c = psum.tile([128, 512], f32, tag="psC",
                                     bufs=2, name=f"psC{tt}_{ot}")
                    for e in range(e0, E):
                        nc.tensor.matmul(
                            ps_c[:],
                            zt[(e, tch)][:, ts(m, 128)],
                            wf_sb[:, e, ts(ot, 512)],
                            start=(e == e0),
                            stop=(e == E - 1),
                        )
                    dst = out_sb[:, ts(ot % 4, 512)]
                    if two_pass:
                        nc.vector.scalar_tensor_tensor(
                            dst, ps_c[:], 1.0, po[(tt, ot)][:],
                            mybir.AluOpType.mult, mybir.AluOpType.add)
                    else:
                        nc.vector.tensor_copy(dst, ps_c[:])
                nc.scalar.dma_start(
                    out_d[ts(tt, 128), O // 2:O], out_sb[:])

    nc.compile()
    return nc


def _prep_inputs(x, weight, bias, combine_array, dispatch_mask):
    """Host-side cast to bf16 + re-layout for contiguous device DMA."""
    x = np.asarray(x, np.float32)
    weight = np.asarray(weight, np.float32)
    bias = np.asarray(bias, np.float32)
    comb = np.asarray(combine_array, np.float32)
    mask = np.asarray(dispatch_mask, np.float32)

    # full-batch layouts (experts 0-3, dup): [B, E, 128, NTT, *]
    xf = np.ascontiguousarray(
        x.reshape(B, NTT, 128, E, I).transpose(0, 3, 2, 1, 4)).astype(BF16)
    mf = np.ascontiguousarray(
        mask.reshape(B, NTT, 128, E, C).transpose(0, 3, 2, 1, 4)
    ).astype(BF16)
    # half-batch layouts (experts 4-7, split): [B, 2, E, 128, NTTL, *]
    xs = np.ascontiguousarray(
        x.reshape(B, 2, NTTL, 128, E, I).transpose(0, 1, 4, 3, 2, 5)
    ).astype(BF16)
    ms = np.ascontiguousarray(
        mask.reshape(B, 2, NTTL, 128, E, C).transpose(0, 1, 4, 3, 2, 5)
    ).astype(BF16)
    # cbt[b,h]: (E, NCT, 128, TLOC)
    cbt = np.ascontiguousarray(
        comb.reshape(B, 2, TLOC, E, NCT, 128).transpose(0, 1, 3, 4, 5, 2)
    ).astype(BF16)
    wfh = np.ascontiguousarray(
        weight.reshape(E, O, I).transpose(2, 0, 1)).astype(BF16)
    s = comb.sum(axis=(2, 3))
    idm = np.eye(128, dtype=BF16)

    in_maps = []
    for k in range(NCORES):
        b, h = k // 2, k % 2
        in_maps.append({
            "xhd": xf[b, 0:4], "mhd": mf[b, 0:4],
            "xhs": xs[b, h, 4:8], "mhs": ms[b, h, 4:8],
            "cbt": cbt[b, h], "wf": wfh, "ident": idm,
        })
    return in_maps, s, bias


def kernel(x, weight, bias, combine_array, dispatch_mask):
    from concourse import bass_utils

    if "nc" not in _CACHE:
        _CACHE["nc"] = _build()
    nc = _CACHE["nc"]

    in_maps, s, bias_f = _prep_inputs(
        x, weight, bias, combine_array, dispatch_mask)
    res = bass_utils.run_bass_kernel_spmd(
        nc, in_maps, core_ids=list(range(NCORES)))
    out = np.stack([res.results[k]["out"] for k in range(NCORES)])
    out = out.reshape(B, T, O)
    out += s[:, :, None] * bias_f[None, None, :]
    return out.astype(np.float32)
